# revision 24
# baseline (speedup 1.0000x reference)
"""Trainium2 Bass kernel for the gnn_message_passing LoopModel.

Reference computation (per edge e, corners l/r from edge_corner):
    CF[n]    = mean over pairs (n, e') of x[e']          (segment mean)
    out[e]   = relu(W1 @ x[e] + W2 @ CF[l_e] + W3 @ CF[r_e] + W4 @ max_e x)

Distribution over 8 NeuronCores:
  - corner table sharded 32 corners/core (host balances incident-pair load);
    scatter stage = dma_gather of incident x rows (fp8) + matmul with a
    host-built scatter matrix (1/count folded in) -> fp8 table slice
    [2048 rows = 32 corners x 64 ch, 1024 cols (784 real hw + pad)]
  - ONE AllGather replicates the fp8 table; global max uses AllReduce-max
  - the ~70us AllGather window is filled with (a) SWDGE prepare_only
    descriptor generation for the phase-4 corner gathers (trigger_dma fires
    them the moment the AllGather lands) and (b) precompute of the
    table-independent conv terms xw = W1 @ x + W4 @ gmax into SBUF
  - conv stage edge-sharded 64 edges/core: 3 accumulating matmuls per PSUM
    tile (W2 @ CF[l] fp8, W3 @ CF[r] fp8, identity @ xw bf16), relu, store
"""

import os
import sys
import numpy as np

for _p in ("/opt/trn_rl_repo", "/root/.axon_site/_ro/trn_rl_repo"):
    if os.path.isdir(_p) and _p not in sys.path:
        sys.path.insert(0, _p)

import ml_dtypes  # noqa: E402
from concourse import bacc, bass, mybir, tile  # noqa: E402
from concourse.bass_utils import run_bass_kernel_spmd  # noqa: E402

N_CORES = 8
E, C, H, W = 512, 64, 28, 28
HW = H * W                      # 784
CH0, CH1 = 512, 272             # hw split: chunk0 cols, chunk1 cols
TCOLS = 1024                    # table row length (784 real + 240 pad)
NC_TOT = 256                    # corner table rows (padded if num_corners < 256)
N_LOC = NC_TOT // N_CORES       # 32 corners per core
E_LOC = E // N_CORES            # 64 edges per core
S0R = N_LOC * C                 # 2048 table rows per core slice

BF16 = ml_dtypes.bfloat16
FP8 = ml_dtypes.float8_e4m3

_PROGRAM_CACHE = {}


# --------------------------------------------------------------------------
# host-side helpers
# --------------------------------------------------------------------------

def _round_bf16(a):
    """fp32 -> bf16 (round to nearest even), returned as uint16."""
    v = np.ascontiguousarray(a, dtype=np.float32).view(np.uint32)
    return ((v + 0x7FFF + ((v >> 16) & 1)) >> 16).astype(np.uint16)


def _balance_corners(counts):
    """Assign NC_TOT corners to N_CORES bins, N_LOC corners per bin,
    minimizing the max total incident-pair count per bin."""
    order = np.argsort(-counts, kind="stable")
    loads = np.zeros(N_CORES, dtype=np.int64)
    slots = np.zeros(N_CORES, dtype=np.int64)
    assign = np.full(NC_TOT, -1, dtype=np.int64)
    for c in order:
        cand = [b for b in range(N_CORES) if slots[b] < N_LOC]
        b = min(cand, key=lambda i: (loads[i], slots[i]))
        assign[c] = b
        loads[b] += counts[c]
        slots[b] += 1
    target = counts.sum() // N_CORES
    for _ in range(4096):
        hi = int(np.argmax(loads))
        lo = int(np.argmin(loads))
        if loads[hi] <= max(target, 128):
            break
        best = None
        ch = np.where(assign == hi)[0]
        cl = np.where(assign == lo)[0]
        for a in ch:
            for b2 in cl:
                d = counts[a] - counts[b2]
                if 0 < d <= loads[hi] - loads[lo]:
                    if best is None or abs(d - (loads[hi] - target)) < abs(
                        best[2] - (loads[hi] - target)
                    ):
                        best = (a, b2, d)
        if best is None:
            break
        a, b2, d = best
        assign[a], assign[b2] = lo, hi
        loads[hi] -= d
        loads[lo] += d
    return assign, loads


def _wrap_idxs(idx_flat, n_pad):
    """Pack flat gather indices into the dma_gather wrapped layout:
    [128, n_pad//16] int16 with logical index i at [i%16, i//16],
    replicated across the 8 groups of 16 partitions."""
    assert n_pad % 16 == 0
    w = np.zeros((16, n_pad // 16), dtype=np.int16)
    for i, v in enumerate(idx_flat):
        w[i % 16, i // 16] = v
    return np.tile(w, (8, 1))


def _prepare(x, W_agg, corner_edge_pairs, edge_corner, num_corners):
    x = np.asarray(x, dtype=np.float32)
    W_agg = np.asarray(W_agg, dtype=np.float32)
    cep = np.asarray(corner_edge_pairs).astype(np.int64)
    ec = np.asarray(edge_corner).astype(np.int64)
    ncorn = int(num_corners)
    assert x.shape == (E, C, H, W), x.shape
    assert ncorn <= NC_TOT

    # reference semantics: scatter drops out-of-range segments, gathers clamp
    seg = cep[:, 0]
    eid = np.clip(cep[:, 1], 0, E - 1)
    valid = (seg >= 0) & (seg < ncorn)
    seg_v, eid_v = seg[valid], eid[valid]
    ec_cl = np.clip(ec, 0, max(ncorn - 1, 0))

    counts = np.bincount(seg_v, minlength=NC_TOT).astype(np.int64)
    inv_count = 1.0 / np.maximum(counts, 1).astype(np.float64)

    assign, loads = _balance_corners(counts)
    k_chunks = max(1, int(-(-int(loads.max()) // 128)))  # ceil(maxload/128)
    k_pad = 128 * k_chunks

    # permuted corner position in the all-gathered table
    pos = np.zeros(NC_TOT, dtype=np.int64)
    slot_ctr = np.zeros(N_CORES, dtype=np.int64)
    for c in range(NC_TOT):
        b = assign[c]
        pos[c] = b * N_LOC + slot_ctr[b]
        slot_ctr[b] += 1

    # per-core incident pair lists
    pair_eids = [[] for _ in range(N_CORES)]
    pair_local = [[] for _ in range(N_CORES)]
    pair_inv = [[] for _ in range(N_CORES)]
    for p in range(len(seg_v)):
        c = int(seg_v[p])
        b = int(assign[c])
        pair_eids[b].append(int(eid_v[p]))
        pair_local[b].append(int(pos[c] - b * N_LOC))
        pair_inv[b].append(inv_count[c])

    # x in bf16 (local conv/max input) and fp8 (scatter gather source)
    xr = x.reshape(E, C, HW)
    xb = _round_bf16(xr)                                  # [E, 64, 784] u16
    x8 = np.clip(xr, -240.0, 240.0).astype(FP8)           # [E, 64, 784] fp8
    # bf16 local layouts (big-tile loads)
    xl0 = np.ascontiguousarray(xb[:, :, :CH0]).reshape(E * 16, 4 * CH0)
    xl1 = np.ascontiguousarray(xb[:, :, CH0:]).reshape(E * 4, 16 * CH1)
    # fp8 gather layouts
    xf0 = np.ascontiguousarray(x8[:, :, :CH0]).reshape(E * 16, 4 * CH0)
    xf1 = np.ascontiguousarray(x8[:, :, CH0:]).reshape(E * 4, 16 * CH1)

    # block-diagonal weights for 2-edge batched conv matmuls
    wblk = np.zeros((4, 128, 128), dtype=np.float32)
    for t in range(4):
        wt = W_agg[:, t * 64:(t + 1) * 64].T          # [c, o]
        wblk[t, :64, :64] = wt
        wblk[t, 64:, 64:] = wt
    wblk_in = _round_bf16(wblk.reshape(4 * 128, 128))
    ident = _round_bf16(np.eye(128, dtype=np.float32))

    per_core = []
    for b in range(N_CORES):
        k_real = len(pair_eids[b])
        assert k_real <= k_pad
        eids_b = np.zeros(k_pad, dtype=np.int64)
        eids_b[:k_real] = pair_eids[b]
        mc = np.zeros((k_pad, N_LOC), dtype=np.float32)
        for p in range(k_real):
            mc[p, pair_local[b][p]] += pair_inv[b][p]

        # stage-1 gather indices
        # chunk0: per (kc, j in 8): 256 idxs, i = s*128 + p, idx = eid*16+2j+s
        # chunk1: per (kc, j4 in 4): 128 idxs, idx = eid*4 + j4
        s10_cols = []
        s11_cols = []
        for kc in range(k_chunks):
            epk = eids_b[kc * 128:(kc + 1) * 128]
            for j in range(8):
                flat = np.zeros(256, dtype=np.int64)
                for s in range(2):
                    flat[s * 128:(s + 1) * 128] = epk * 16 + (2 * j + s)
                s10_cols.append(_wrap_idxs(flat.astype(np.int16), 256))
            for j4 in range(4):
                s11_cols.append(_wrap_idxs((epk * 4 + j4).astype(np.int16), 128))
        s10 = np.concatenate(s10_cols, axis=1)   # [128, 16*8*k_chunks]
        s11 = np.concatenate(s11_cols, axis=1)   # [128, 8*4*k_chunks]

        # stage-4 gather indices: per binstr, 1024 idxs into the ag table
        # i = s*128 + m*64 + ch -> table row of corner(edge, t), channel ch
        e0 = b * E_LOC
        s4_cols = []
        ch64 = np.arange(64, dtype=np.int64)
        for binstr in range(8):
            f = np.zeros(1024, dtype=np.int64)
            for ep in range(4):
                for t in range(2):
                    s = ep * 2 + t
                    for m in range(2):
                        le = binstr * 8 + ep * 2 + m
                        p_c = pos[int(ec_cl[e0 + le, t])]
                        ob, sl = p_c // N_LOC, p_c % N_LOC
                        i0 = s * 128 + m * 64
                        f[i0:i0 + 64] = ob * S0R + sl * 64 + ch64
            s4_cols.append(_wrap_idxs(f.astype(np.int16), 1024))
        s4 = np.concatenate(s4_cols, axis=1)     # [128, 512]

        per_core.append(dict(mc=_round_bf16(mc), s10=s10, s11=s11, s4=s4))

    return xl0, xl1, xf0, xf1, wblk_in, ident, per_core, k_chunks


# --------------------------------------------------------------------------
# device program
# --------------------------------------------------------------------------

def _build_program(k_chunks):
    dbg_no_prep = os.environ.get("DBG_NO_PREP", "0") == "1"
    dbg_zero_table = os.environ.get("DBG_ZERO_TABLE", "0") == "1"
    dbg_zero_xw = os.environ.get("DBG_ZERO_XW", "0") == "1"
    bf = mybir.dt.bfloat16
    f8 = mybir.dt.float8e4
    f32 = mybir.dt.float32
    i16 = mybir.dt.int16

    nc = bacc.Bacc("TRN2", target_bir_lowering=False, debug=False,
                   num_devices=N_CORES, num_swdge_queues=2)

    xf0_t = nc.dram_tensor("xf0", [E * 16, 4 * CH0], f8, kind="ExternalInput").ap()
    xf1_t = nc.dram_tensor("xf1", [E * 4, 16 * CH1], f8, kind="ExternalInput").ap()
    xl0_t = nc.dram_tensor("xl0", [E_LOC * 16, 4 * CH0], bf, kind="ExternalInput").ap()
    xl1_t = nc.dram_tensor("xl1", [E_LOC * 4, 16 * CH1], bf, kind="ExternalInput").ap()
    wb_t = nc.dram_tensor("wb", [4 * 128, 128], bf, kind="ExternalInput").ap()
    id_t = nc.dram_tensor("ident", [128, 128], bf, kind="ExternalInput").ap()
    mc_t = nc.dram_tensor("mc", [128 * k_chunks, N_LOC], bf, kind="ExternalInput").ap()
    s10_t = nc.dram_tensor("s10", [128, 16 * 8 * k_chunks], i16, kind="ExternalInput").ap()
    s11_t = nc.dram_tensor("s11", [128, 8 * 4 * k_chunks], i16, kind="ExternalInput").ap()
    s4_t = nc.dram_tensor("s4", [128, 512], i16, kind="ExternalInput").ap()
    out_t = nc.dram_tensor("outp", [E_LOC * C, HW], bf, kind="ExternalOutput").ap()
    dbg_dump = os.environ.get("DBG_DUMP_TABLE", "0") != "0"
    if dbg_dump:
        dump_t = nc.dram_tensor("dump", [N_CORES * S0R, TCOLS], mybir.dt.uint8,
                                kind="ExternalOutput").ap()

    with tile.TileContext(nc) as tc:
        with tc.tile_pool(name="dram", bufs=1, space="DRAM") as dram, \
             tc.tile_pool(name="consts", bufs=1) as consts, \
             tc.tile_pool(name="xw", bufs=1) as xwp:
            slice_t = dram.tile([S0R, TCOLS], f8)
            ag_t = dram.tile([N_CORES * S0R, TCOLS], f8, addr_space="Shared")
            slice_g = dram.tile([C, HW], bf)
            agg = dram.tile([C, HW], bf, addr_space="Shared")

            # constants (one DMA each)
            wtile = consts.tile([128, 4, 128], bf, tag="wt")
            nc.sync.dma_start(out=wtile[:],
                              in_=wb_t[:].rearrange("(t p) o -> p t o", t=4))
            itile = consts.tile([128, 128], bf, tag="id")
            nc.sync.dma_start(out=itile[:], in_=id_t[:])
            mctiles = []
            for kc in range(k_chunks):
                mt = consts.tile([128, N_LOC], bf, tag=f"mc{kc}")
                nc.sync.dma_start(out=mt[:], in_=mc_t[kc * 128:(kc + 1) * 128, :])
                mctiles.append(mt)
            s10tile = consts.tile([128, 16 * 8 * k_chunks], i16, tag="s10")
            nc.sync.dma_start(out=s10tile[:], in_=s10_t[:])
            s11tile = consts.tile([128, 8 * 4 * k_chunks], i16, tag="s11")
            nc.sync.dma_start(out=s11tile[:], in_=s11_t[:])
            s4tile = consts.tile([128, 512], i16, tag="s4")
            nc.sync.dma_start(out=s4tile[:], in_=s4_t[:])
            gm0 = consts.tile([128, CH0], bf, tag="gm0")
            gm1 = consts.tile([128, CH1], bf, tag="gm1")

            # zero the table pad columns (cols HW..TCOLS) once
            PAD = TCOLS - HW
            zt = consts.tile([128, (S0R // 128) * PAD], mybir.dt.float8e4,
                             tag="zt")
            nc.vector.memset(zt[:], 0.0)
            nc.sync.dma_start(
                out=slice_t[:].rearrange("(g r) d -> r g d", r=128)
                [:, :, HW:TCOLS],
                in_=zt[:],
            )

            # xw: starts as W1 @ x, then += W4 @ gmax (vector add during AG)
            xw0 = xwp.tile([128, 32 * CH0], bf, tag="xw0")
            xw1 = xwp.tile([128, 32 * CH1], bf, tag="xw1")
            gt0 = consts.tile([128, CH0], bf, tag="gt0")
            gt1 = consts.tile([128, CH1], bf, tag="gt1")

            def w(t):
                return wtile[:, t, :]

            # phase-4 gather tiles: own up-front pool so the prepped DMA
            # writes never land in SBUF space being reused by other pools
            lrts = []
            for binstr in range(8):
                lrt = xwp.tile([128, 8, TCOLS], f8, tag=f"lrt{binstr}")
                lrts.append(lrt)

            with tc.tile_pool(name="px", bufs=3) as px, \
                 tc.tile_pool(name="p2", bufs=4) as p2, \
                 tc.tile_pool(name="psx", bufs=2, space="PSUM") as psx, \
                 tc.tile_pool(name="p1", bufs=4) as p1, \
                 tc.tile_pool(name="p1s", bufs=4) as p1s, \
                 tc.tile_pool(name="psum1", bufs=2, space="PSUM") as psum1:
                mx0 = p2.tile([128, CH0], bf, tag="mx0")
                mx1 = p2.tile([128, CH1], bf, tag="mx1")

                def stream_x(cc, chunk):
                    CH = CH0 if chunk == 0 else CH1
                    a = 4 if chunk == 0 else 16
                    xlt = xl0_t if chunk == 0 else xl1_t
                    xwt = xw0 if chunk == 0 else xw1
                    mx = mx0 if chunk == 0 else mx1
                    xc = px.tile([128, 8 * CH], bf, tag=f"xc{chunk}")
                    nc.sync.dma_start(
                        out=xc[:].rearrange("p (j d) -> p j d", j=8),
                        in_=xlt[:].rearrange("(j r) (a d) -> (r a) j d",
                                             j=32, a=a)[:, 8 * cc:8 * cc + 8, :],
                    )
                    for jj in range(8):
                        j = cc * 8 + jj
                        sl = xc[:, jj * CH:(jj + 1) * CH]
                        if j == 0:
                            nc.vector.tensor_copy(out=mx[:], in_=sl)
                        else:
                            nc.vector.tensor_tensor(out=mx[:], in0=mx[:],
                                                    in1=sl,
                                                    op=mybir.AluOpType.max)
                        ps = psx.tile([128, CH], f32, space="PSUM",
                                      tag=f"psx{chunk}")
                        nc.tensor.matmul(out=ps[:], lhsT=w(0), rhs=sl,
                                         start=True, stop=True)
                        if j % 2 == 0:
                            nc.scalar.activation(xwt[:, j * CH:(j + 1) * CH],
                                                 ps[:],
                                                 mybir.ActivationFunctionType.Copy)
                        else:
                            nc.vector.tensor_copy(
                                out=xwt[:, j * CH:(j + 1) * CH], in_=ps[:])

                def scatter0(j):
                    gts = []
                    for kc in range(k_chunks):
                        gt = p1.tile([128, 2, 4 * CH0], f8, tag="gt0")
                        nc.gpsimd.dma_gather(
                            gt[:], xf0_t[:],
                            s10tile[:, (kc * 8 + j) * 16:(kc * 8 + j) * 16 + 16],
                            num_idxs=256, num_idxs_reg=256, elem_size=4 * CH0,
                        )
                        gts.append(gt)
                    for t in range(2):
                        ps = psum1.tile([128, CH0], f32, space="PSUM", tag="ps1a")
                        for i in range(4):
                            q = t * 4 + i
                            sl, chp = q // 4, q % 4
                            for kc in range(k_chunks):
                                nc.tensor.matmul(
                                    out=ps[32 * i:32 * (i + 1), :],
                                    lhsT=mctiles[kc][:],
                                    rhs=gts[kc][:, sl, chp * CH0:(chp + 1) * CH0],
                                    start=(kc == 0), stop=(kc == k_chunks - 1),
                                    tile_position=(0, 32 * i),
                                )
                        stg = p1s.tile([128, CH0], f8, tag="stg0")
                        if t == 0:
                            nc.scalar.activation(stg[:], ps[:],
                                                 mybir.ActivationFunctionType.Copy)
                        else:
                            nc.vector.tensor_copy(out=stg[:], in_=ps[:])
                        nc.sync.dma_start(
                            out=slice_t[:]
                            .rearrange("(c h) d -> h c d", c=N_LOC)
                            [8 * j + 4 * t:8 * j + 4 * t + 4, :, 0:CH0],
                            in_=stg[:],
                        )

                def scatter1(j4):
                    gts = []
                    for kc in range(k_chunks):
                        gt = p1.tile([128, 1, 16 * CH1], f8, tag="gt1")
                        nc.gpsimd.dma_gather(
                            gt[:], xf1_t[:],
                            s11tile[:, (kc * 4 + j4) * 8:(kc * 4 + j4) * 8 + 8],
                            num_idxs=128, num_idxs_reg=128, elem_size=16 * CH1,
                        )
                        gts.append(gt)
                    for t in range(4):
                        ps = psum1.tile([128, CH1], f32, space="PSUM", tag="ps1b")
                        for i in range(4):
                            q = t * 4 + i
                            for kc in range(k_chunks):
                                nc.tensor.matmul(
                                    out=ps[32 * i:32 * (i + 1), :],
                                    lhsT=mctiles[kc][:],
                                    rhs=gts[kc][:, 0, q * CH1:(q + 1) * CH1],
                                    start=(kc == 0), stop=(kc == k_chunks - 1),
                                    tile_position=(0, 32 * i),
                                )
                        stg = p1s.tile([128, CH1], f8, tag="stg1")
                        if t % 2 == 0:
                            nc.scalar.activation(stg[:], ps[:],
                                                 mybir.ActivationFunctionType.Copy)
                        else:
                            nc.vector.tensor_copy(out=stg[:], in_=ps[:])
                        nc.sync.dma_start(
                            out=slice_t[:]
                            .rearrange("(c h) d -> h c d", c=N_LOC)
                            [16 * j4 + 4 * t:16 * j4 + 4 * t + 4, :,
                             CH0:CH0 + CH1],
                            in_=stg[:],
                        )

                # interleave x streaming with scatter work
                for cc in range(4):
                    stream_x(cc, 0)
                    scatter0(2 * cc)
                    scatter0(2 * cc + 1)
                for cc in range(4):
                    stream_x(cc, 1)
                    scatter1(cc)

                # ---- local max fold + AllReduce-max -----------------------
                half0 = p2.tile([64, CH0], bf, tag="h0")
                nc.sync.dma_start(out=half0[:], in_=mx0[64:128, :])
                nc.vector.tensor_tensor(out=mx0[0:64, :], in0=mx0[0:64, :],
                                        in1=half0[:], op=mybir.AluOpType.max)
                half1 = p2.tile([64, CH1], bf, tag="h1")
                nc.sync.dma_start(out=half1[:], in_=mx1[64:128, :])
                nc.vector.tensor_tensor(out=mx1[0:64, :], in0=mx1[0:64, :],
                                        in1=half1[:], op=mybir.AluOpType.max)
                nc.sync.dma_start(out=slice_g[:, 0:CH0], in_=mx0[0:64, :])
                nc.sync.dma_start(out=slice_g[:, CH0:HW], in_=mx1[0:64, :])

                # ---- collectives ------------------------------------------
                nc.gpsimd.collective_compute(
                    "AllReduce", mybir.AluOpType.max,
                    replica_groups=[list(range(N_CORES))],
                    ins=[slice_g.opt()], outs=[agg.opt()],
                )
                nc.gpsimd.collective_compute(
                    "AllGather", mybir.AluOpType.bypass,
                    replica_groups=[list(range(N_CORES))],
                    ins=[slice_t.opt()], outs=[ag_t.opt()],
                )

                # global max into 2-edge-stacked tiles
                nc.sync.dma_start(out=gm0[0:64, :], in_=agg[:, 0:CH0])
                nc.sync.dma_start(out=gm0[64:128, :], in_=agg[:, 0:CH0])
                nc.sync.dma_start(out=gm1[0:64, :], in_=agg[:, CH0:HW])
                nc.sync.dma_start(out=gm1[64:128, :], in_=agg[:, CH0:HW])

            # ---- g-term (fills the AllGather window) ----------------------
            with tc.tile_pool(name="psumw", bufs=2, space="PSUM") as psumw:
                psg0 = psumw.tile([128, CH0], f32, space="PSUM", tag="psg0")
                nc.tensor.matmul(out=psg0[:], lhsT=w(3), rhs=gm0[:],
                                 start=True, stop=True)
                nc.scalar.activation(gt0[:], psg0[:],
                                     mybir.ActivationFunctionType.Copy)
                psg1 = psumw.tile([128, CH1], f32, space="PSUM", tag="psg1")
                nc.tensor.matmul(out=psg1[:], lhsT=w(3), rhs=gm1[:],
                                 start=True, stop=True)
                nc.scalar.activation(gt1[:], psg1[:],
                                     mybir.ActivationFunctionType.Copy)
            for j in range(32):
                nc.vector.tensor_tensor(
                    out=xw0[:, j * CH0:(j + 1) * CH0],
                    in0=xw0[:, j * CH0:(j + 1) * CH0], in1=gt0[:],
                    op=mybir.AluOpType.add)
            for j in range(32):
                nc.vector.tensor_tensor(
                    out=xw1[:, j * CH1:(j + 1) * CH1],
                    in0=xw1[:, j * CH1:(j + 1) * CH1], in1=gt1[:],
                    op=mybir.AluOpType.add)

            if dbg_dump:
                if os.environ.get("DBG_DUMP_TABLE") == "2":
                    nc.sync.dma_start(out=dump_t[0:S0R, :],
                                      in_=slice_t[:].bitcast(mybir.dt.uint8))
                else:
                    nc.sync.dma_start(out=dump_t[:],
                                      in_=ag_t[:].bitcast(mybir.dt.uint8))

            # ---- phase 4: prep gathers during AG, trigger after -----------
            with tc.tile_pool(name="p4o", bufs=4) as p4o, \
                 tc.tile_pool(name="psum4", bufs=4, space="PSUM") as psum4:
                p4sems = []
                for binstr in range(8):
                    lrt = lrts[binstr]
                    if dbg_no_prep:
                        nc.gpsimd.dma_gather(
                            lrt[:], ag_t[:],
                            s4tile[:, binstr * 64:(binstr + 1) * 64],
                            num_idxs=1024, num_idxs_reg=1024, elem_size=TCOLS,
                        )
                        p4sems.append(None)
                    else:
                        dma_sem = nc.alloc_semaphore(f"p4g{binstr}")
                        nc.gpsimd.dma_gather(
                            lrt[:], ag_t[:],
                            s4tile[:, binstr * 64:(binstr + 1) * 64],
                            num_idxs=1024, num_idxs_reg=1024, elem_size=TCOLS,
                            prepare_only=True, sem=dma_sem, queue_num=1,
                        )
                        p4sems.append(dma_sem)
                if not dbg_no_prep:
                    nc.gpsimd.trigger_dma(count=None, queue_num=1)

                for binstr in range(8):
                    lrt = lrts[binstr]
                    if p4sems[binstr] is not None:
                        nc.tensor.wait_ge(p4sems[binstr], 16)
                    ot0 = p4o.tile([128, 4, CH0], bf, tag="ot0")
                    ot1 = p4o.tile([128, 4, CH1], bf, tag="ot1")
                    for ep in range(4):
                        je = binstr * 4 + ep
                        mm0 = []
                        if not dbg_zero_table:
                            mm0 += [(w(1), lambda l=lrt, s=ep * 2: l[:, s, 0:CH0]),
                                    (w(2), lambda l=lrt, s=ep * 2 + 1: l[:, s, 0:CH0])]
                        if not dbg_zero_xw:
                            mm0 += [(itile[:], lambda j=je: xw0[:, j * CH0:(j + 1) * CH0])]
                        ps0 = psum4.tile([128, CH0], f32, space="PSUM", tag="ps40")
                        for k, (lh, rh) in enumerate(mm0):
                            nc.tensor.matmul(out=ps0[:], lhsT=lh, rhs=rh(),
                                             start=(k == 0), stop=(k == len(mm0) - 1))
                        nc.scalar.activation(ot0[:, ep, :], ps0[:],
                                             mybir.ActivationFunctionType.Relu)
                        mm1 = []
                        if not dbg_zero_table:
                            mm1 += [(w(1), lambda l=lrt, s=ep * 2: l[:, s, CH0:CH0 + CH1]),
                                    (w(2), lambda l=lrt, s=ep * 2 + 1: l[:, s, CH0:CH0 + CH1])]
                        if not dbg_zero_xw:
                            mm1 += [(itile[:], lambda j=je: xw1[:, j * CH1:(j + 1) * CH1])]
                        ps1 = psum4.tile([128, CH1], f32, space="PSUM", tag="ps41")
                        for k, (lh, rh) in enumerate(mm1):
                            nc.tensor.matmul(out=ps1[:], lhsT=lh, rhs=rh(),
                                             start=(k == 0), stop=(k == len(mm1) - 1))
                        nc.scalar.activation(ot1[:, ep, :], ps1[:],
                                             mybir.ActivationFunctionType.Relu)
                    # out row (binstr*8 + ep*2)*64 + p = bi*512 + ep*128 + p
                    nc.sync.dma_start(
                        out=out_t[:]
                        .rearrange("(bi ep p) d -> bi p ep d", bi=8, ep=4)
                        [binstr, :, :, 0:CH0],
                        in_=ot0[:],
                    )
                    nc.sync.dma_start(
                        out=out_t[:]
                        .rearrange("(bi ep p) d -> bi p ep d", bi=8, ep=4)
                        [binstr, :, :, CH0:HW],
                        in_=ot1[:],
                    )

    nc.compile()
    return nc


# --------------------------------------------------------------------------
# entry point
# --------------------------------------------------------------------------

def _run(x, W_agg, corner_edge_pairs, edge_corner, num_corners,
         trace=False):
    xl0, xl1, xf0, xf1, wblk_in, ident, per_core, k_chunks = _prepare(
        x, W_agg, corner_edge_pairs, edge_corner, num_corners)

    key = (k_chunks, os.environ.get("DBG_NO_PREP"),
           os.environ.get("DBG_ZERO_TABLE"), os.environ.get("DBG_ZERO_XW"))
    if key not in _PROGRAM_CACHE:
        _PROGRAM_CACHE[key] = _build_program(k_chunks)
    nc = _PROGRAM_CACHE[key]

    xl0_b = xl0.view(BF16)
    xl1_b = xl1.view(BF16)
    in_maps = []
    for b in range(N_CORES):
        pc = per_core[b]
        in_maps.append({
            "xf0": xf0, "xf1": xf1,
            "xl0": xl0_b[b * E_LOC * 16:(b + 1) * E_LOC * 16],
            "xl1": xl1_b[b * E_LOC * 4:(b + 1) * E_LOC * 4],
            "wb": wblk_in.view(BF16), "ident": ident.view(BF16),
            "mc": pc["mc"].view(BF16),
            "s10": pc["s10"], "s11": pc["s11"], "s4": pc["s4"],
        })

    kwargs = {}
    if trace:
        kwargs = dict(trace=True, trace_cores=list(range(N_CORES)))
    res = run_bass_kernel_spmd(nc, in_maps, list(range(N_CORES)), **kwargs)

    out = np.empty((E, C, HW), dtype=np.float32)
    for b in range(N_CORES):
        o = np.asarray(res.results[b]["outp"]).view(np.uint16)
        f = (o.astype(np.uint32) << 16).view(np.float32).reshape(E_LOC, C, HW)
        out[b * E_LOC:(b + 1) * E_LOC] = f
    return out.reshape(E, C, H, W), res


def kernel(x, W_agg, corner_edge_pairs, edge_corner, num_corners):
    out, _ = _run(x, W_agg, corner_edge_pairs, edge_corner, num_corners,
                  trace=False)
    return out


# expose for test harness profiling
def _run_profiled(x, W_agg, corner_edge_pairs, edge_corner, num_corners,
                  trace=True):
    return _run(x, W_agg, corner_edge_pairs, edge_corner, num_corners,
                trace=trace)
